# revision 19
# baseline (speedup 1.0000x reference)
"""Trainium2 Bass kernel for nn_CPADConvOffsetStage.

The reference module is:
  up_posi = grid_sample_bilinear_border(posi_map -> [B,16,GP,GP], grid = base + offset*scale)
  h       = relu(w1 @ up_posi + b1)           (1x1 conv)
  weights = (w2 @ h + b2).reshape(B,64,9,H,W) (1x1 conv -> per-pixel 3x3 kernels)
  x_adapt = w_ca @ x                          (1x1 conv)
  out     = sum_k weights[:,:,k] * unfold3x3(x_adapt)[:,:,k] + bias

In setup_inputs() posi_map is spatially constant per channel (jnp.ones).
Bilinear interpolation weights sum to exactly 1, so up_posi is spatially
constant => h, weights are spatially constant => the whole module reduces
to ONE dense 3x3 convolution with host-precomputable weights
    Wfull[o,c,k] = wk[o,k] * w_ca[o,c],   wk = (w2 @ relu(w1 @ v + b1) + b2)
plus the bias.  The kernel below runs that conv data-parallel over batch
(1 batch image per NeuronCore, 8 cores).

If posi_map is NOT per-channel spatially constant (never the case for the
shipped setup_inputs), we fall back to an exact numpy port of the
reference.
"""

import os
import numpy as np
from contextlib import ExitStack

import concourse.bass as bass
import concourse.tile as tile
from concourse import mybir
from concourse.bass_utils import run_bass_kernel_spmd

# Problem constants (hardcoded per contract)
B, C, H, W = 8, 64, 128, 128
OC = 64
KK = 3
POSI_CH, GP = 16, 16
NCORES = 8
F32 = mybir.dt.float32

HPAD, WPAD = H + 2, W + 2      # host-padded image (130 x 130)
ROWS_PER_TILE = 4              # 4 rows * 128 cols = 512 = max fp32 moving free dim
NFREE = ROWS_PER_TILE * W      # 512
RB = 16                        # output rows per SBUF block
NBLK = H // RB                 # blocks per image
SUB = RB // ROWS_PER_TILE      # psum tiles per block
NXB = 4                        # rotated input block buffers
F32R = mybir.dt.float32r       # fp32 storage, single-pass relaxed-precision matmul

_cached_nc = None
_cached_variant = None
last_results = None            # test harness introspection


def _ensure_ntff_hook():
    """Register the axon NTFF-profile hook that this image's antenv lacks.

    run_bass_kernel_spmd(trace=True) under axon needs
    antenv.axon_hooks.get_axon_ntff_profile_hook; the hook machinery
    exists in trn_agent_boot but was never registered because
    antenv.axon_hooks is missing.  Recreate the module in sys.modules.
    """
    import sys
    import types

    if "antenv.axon_hooks" in sys.modules:
        return
    try:
        from trn_agent_boot.trn_boot import _ntff_profile_via_ctypes

        hook = _ntff_profile_via_ctypes("/opt/axon/libaxon_pjrt.so")
    except Exception:
        hook = None
    mod = types.ModuleType("antenv.axon_hooks")
    mod.get_axon_ntff_profile_hook = lambda: hook
    mod.set_axon_ntff_profile_hook = lambda h: None
    sys.modules["antenv.axon_hooks"] = mod
    try:
        import antenv

        antenv.axon_hooks = mod
    except Exception:
        pass


def _build_conv_nc(variant="bf16_tiled", split_waits=True):
    """3x3 conv, 64->64 ch, on one (host-padded) [64,130,130] image.

    SPMD over 8 cores, one batch image per core.  The input arrives
    zero-padded from the host so every 3x3 tap is a plain shifted
    window read; no memsets / halo special cases on device (this also
    keeps every Matmult at <=2 semaphore waits — walrus rejects 3+ on
    the LDWEIGHTS struct).
    """
    o16 = variant.endswith("_o16")
    base_variant = variant[:-4] if o16 else variant
    tiled = base_variant in ("bf16_tiled", "bf16_dup3")
    dup = base_variant == "bf16_dup3"
    mm_dt = mybir.dt.bfloat16 if tiled else F32R
    o_dt = mybir.dt.bfloat16 if o16 else F32
    xb_parts = 128 if dup else C
    w_cols = 6 * OC if dup else 9 * OC
    nc = bass.Bass()
    x_d = nc.declare_dram_parameter("x", [C, HPAD, WPAD], mm_dt, isOutput=False)
    w_d = nc.declare_dram_parameter("wts", [xb_parts, w_cols], mm_dt, isOutput=False)
    b_d = nc.declare_dram_parameter("wb", [OC, 1], F32, isOutput=False)
    o_d = nc.declare_dram_parameter("out", [OC, H, W], o_dt, isOutput=True)

    with ExitStack() as ctx:
        tc = ctx.enter_context(tile.TileContext(nc))
        singles = ctx.enter_context(tc.tile_pool(name="singles", bufs=1))
        outs = ctx.enter_context(tc.tile_pool(name="outs", bufs=4))
        psum = ctx.enter_context(
            tc.tile_pool(name="psum", bufs=8 if dup else 4, space="PSUM")
        )
        tmps = ctx.enter_context(tc.tile_pool(name="tmps", bufs=4))

        w_sb = singles.tile([xb_parts, w_cols], mm_dt)
        nc.sync.dma_start(out=w_sb[:, :], in_=w_d[:, :])
        b_sb = singles.tile([OC, 1], F32)
        nc.sync.dma_start(out=b_sb[:, :], in_=b_d[:, :])

        # Rotated input blocks: 18 padded rows each (16 output rows + halo)
        xbs = []
        for i in range(NXB):
            xb_buf = singles.tile([xb_parts, RB + 2, WPAD], mm_dt, tag=f"xb{i}", name=f"xb{i}")
            if dup:
                # The duplicate (shifted) half never receives its last
                # column from DMA; zero it once so K=128 solo matmuls
                # (whose bottom-half weights are zero) can't hit NaN*0.
                nc.vector.memset(xb_buf[C : 2 * C, :, WPAD - 1 : WPAD], 0.0)
            xbs.append(xb_buf)

        # Tap k -> column group: A (psum partitions 0:64, tile_position (0,0))
        # gets taps 0,2,4,6,8; B (64:128, (0,64)) gets 1,3,5,7.  The two
        # column groups of the 128x128 PE array run concurrently (separate
        # XBUS streams), so 9 taps cost ~5 matmul slots instead of 9.
        for blk in range(NBLK):
            xb = xbs[blk % NXB]
            # Prefetch the input pipeline aggressively: schedule each block's
            # load + shifted-duplicate copy as if issued a block earlier, so
            # the first matmul of a block never waits on them (the profile
            # showed ~2.5us PE gaps at early block boundaries).
            with tc.high_priority(offset=60):
                nc.sync.dma_start(
                    out=xb[0:C, :, :],
                    in_=x_d[:, blk * RB : blk * RB + RB + 2, :],
                )
                if dup:
                    # partitions 64:128 = same rows shifted one column left,
                    # so a K=128 matmul contracts two horizontally-adjacent
                    # taps at once.  Derived on-chip (cross-partition copy)
                    # instead of a second DRAM read — DMA is the bottleneck.
                    nc.vector.tensor_copy(
                        xb[C : 2 * C, :, 0 : WPAD - 1],
                        xb[0:C, :, 1:WPAD],
                    )
            o_blk = outs.tile([OC, RB * W], o_dt)
            for s in range(SUB):
                r0 = s * ROWS_PER_TILE        # row offset within block
                if dup:
                    ps = psum.tile([128, NFREE], F32)
                    # 6 all-K=128 matmuls -> 3 col-tiled slots (K=128 streams
                    # ~2x faster per column than K=64 on this part, and the
                    # solo taps' bottom-half weights are zero):
                    #  A: pair(0,1) | pair(6,7) | solo k5
                    #  B: pair(3,4) | solo k2   | solo k8
                    mms = [
                        (0, 0, 0, True, False),   # pair row 0 -> A
                        (1, 1, 1, True, False),   # pair row 1 -> B
                        (2, 2, 0, False, False),  # pair row 2 -> A
                        (3, 0, 1, False, False),  # k2 (row 0, col 2) -> B
                        (4, 1, 0, False, True),   # k5 (row 1, col 2) -> A
                        (5, 2, 1, False, True),   # k8 (row 2, col 2) -> B
                    ]
                    for wi, i, col, st, sp in mms:
                        j = 0 if wi < 3 else 2
                        rhs = xb[:, r0 + i : r0 + i + ROWS_PER_TILE, j : j + W]
                        nc.tensor.matmul(
                            ps[64 * col : 64 * col + OC, :],
                            lhsT=w_sb[:, wi * OC : (wi + 1) * OC],
                            rhs=rhs,
                            start=st,
                            stop=sp,
                            tile_position=(0, 64 * col),
                            skip_group_check=True,
                        )
                elif tiled:
                    ps = psum.tile([128, NFREE], F32)
                    for k in range(9):
                        i, j = divmod(k, 3)
                        rhs = xb[:, r0 + i : r0 + i + ROWS_PER_TILE, j : j + W]
                        col = k % 2           # even taps -> A, odd -> B
                        nc.tensor.matmul(
                            ps[64 * col : 64 * col + OC, :],
                            lhsT=w_sb[:, k * OC : (k + 1) * OC],
                            rhs=rhs,
                            start=(k < 2),
                            stop=(k >= 7),
                            tile_position=(0, 64 * col),
                            skip_group_check=True,
                        )
                if tiled:
                    # Fold: B half evicted by ACT (with bias), A half added
                    # by DVE (walrus allows only one PSUM input per DVE op).
                    ps_a = ps[0:OC, 0:NFREE]
                    ps_b = ps[64:128, 0:NFREE]
                    tmpb = tmps.tile([OC, NFREE], F32)
                    nc.scalar.activation(
                        out=tmpb[:, :],
                        in_=ps_b,
                        func=mybir.ActivationFunctionType.Identity,
                        bias=b_sb[:, 0:1],
                        scale=1.0,
                    )
                    nc.vector.tensor_add(
                        o_blk[:, s * NFREE : (s + 1) * NFREE],
                        ps_a,
                        tmpb[:, :],
                    )
                else:
                    ps = psum.tile([OC, NFREE], F32)
                    for k in range(9):
                        i, j = divmod(k, 3)
                        rhs = xb[:, r0 + i : r0 + i + ROWS_PER_TILE, j : j + W]
                        nc.tensor.matmul(
                            ps[:, :],
                            lhsT=w_sb[:, k * OC : (k + 1) * OC],
                            rhs=rhs,
                            start=(k == 0),
                            stop=(k == 8),
                        )
                    nc.scalar.activation(
                        out=o_blk[:, s * NFREE : (s + 1) * NFREE],
                        in_=ps[:, :],
                        func=mybir.ActivationFunctionType.Identity,
                        bias=b_sb[:, 0:1],
                        scale=1.0,
                    )
            # Two half-block output DMAs: the first half drains while the
            # second half is still being computed (shorter kernel tail).
            HRB = RB // 2
            for h in range(2):
                nc.sync.dma_start(
                    out=o_d[:, blk * RB + h * HRB : blk * RB + (h + 1) * HRB, :],
                    in_=o_blk[:, h * HRB * W : (h + 1) * HRB * W].rearrange(
                        "p (r w) -> p r w", r=HRB
                    ),
                )
    if split_waits:
        _split_sync_waits(nc)
    return nc


def _split_sync_waits(nc, limit=1):
    """Hoist extra sync waits onto injected wait-only EventSemaphore ops.

    The neuronxcc walrus used under axon rejects compute instructions
    carrying more than one sync wait ("Too many sync wait commands", e.g.
    S3_LW / S3D3_AC structs).  Tile's sem assignment emits up to ~3.
    For every instruction with >limit waits, keep the first `limit` and
    prepend one wait-only EventSemaphore per extra wait on the same
    engine (same program position => same semantics).
    """
    import copy as _copy

    f = nc.m.functions[0]
    template = None
    for blk in f.blocks:
        for inst in blk.instructions:
            if type(inst).__name__ == "InstEventSemaphore":
                template = inst
                break
        if template is not None:
            break
    if template is None:
        return
    n_split = 0
    for blk in f.blocks:
        new_list = []
        changed = False
        for inst in blk.instructions:
            si = getattr(inst, "sync_info", None)
            ilimit = limit
            waits = list(si.on_wait) if (si and si.on_wait) else []
            if len(waits) > ilimit:
                for w in waits[ilimit:]:
                    ev = _copy.deepcopy(template)
                    ev.name = f"waitsplit_{n_split}"
                    n_split += 1
                    ev.engine = inst.engine
                    ev.sync_info = mybir.SyncInfo(on_wait=[w], on_update=[])
                    new_list.append(ev)
                inst.sync_info = mybir.SyncInfo(
                    on_wait=waits[:ilimit], on_update=list(si.on_update or [])
                )
                changed = True
            new_list.append(inst)
        if changed:
            blk.instructions = new_list


ROW_CHUNKS = [(0, 18), (18, 50), (50, 82), (82, 114), (114, 130)]


def _build_conv_nc_v2(split_waits=True):
    """SBUF-resident 3x3 conv, 64->64 ch, one padded [64,130,130] image/core.

    Differences vs _build_conv_nc(bf16_dup3):
      * The whole padded input lives in one SBUF tile (partitions 0:64 =
        x, 64:128 = one-column-left-shifted dup), loaded by 5 chunked
        DMAs + per-chunk shift copies -- no per-block halo re-reads and
        the first matmul only waits on an 18-row chunk.
      * The two PE column groups compute DISJOINT row ranges (A = rows
        0:8 of each 16-row block, B = rows 8:16) instead of partial
        sums of the same pixels, so the per-subtile PSUM fold
        (Scalar IDENTITY + Vector ADD) disappears entirely; each PSUM
        half is evicted once, with the bias, directly to bf16.
      * Evictions alternate Vector (A half, tensor_scalar_add) and
        Scalar (B half, activation+bias) so neither engine is a
        serial bottleneck.
      * Output is a single SBUF-resident bf16 image written back by 4
        large DMAs (one per 2 blocks) -- ~4x fewer, larger output
        descriptors than the fp32 half-block DMAs.
    """
    nc = bass.Bass()
    x_d = nc.declare_dram_parameter("x", [C, HPAD, WPAD], mybir.dt.bfloat16,
                                    isOutput=False)
    w_d = nc.declare_dram_parameter("wts", [128, 6 * OC], mybir.dt.bfloat16,
                                    isOutput=False)
    b_d = nc.declare_dram_parameter("wb", [OC, 1], F32, isOutput=False)
    o_d = nc.declare_dram_parameter("out", [OC, H, W], mybir.dt.bfloat16,
                                    isOutput=True)

    with ExitStack() as ctx:
        tc = ctx.enter_context(tile.TileContext(nc))
        singles = ctx.enter_context(tc.tile_pool(name="singles", bufs=1))
        psum = ctx.enter_context(tc.tile_pool(name="psum", bufs=8, space="PSUM"))

        w_sb = singles.tile([128, 6 * OC], mybir.dt.bfloat16)
        b_sb = singles.tile([OC, 1], F32)
        x_sb = singles.tile([128, HPAD, WPAD], mybir.dt.bfloat16)
        o_sb = singles.tile([OC, H * W], mybir.dt.bfloat16)

        with tc.high_priority(offset=60):
            nc.sync.dma_start(out=w_sb[:, :], in_=w_d[:, :])
            nc.sync.dma_start(out=b_sb[:, :], in_=b_d[:, :])
            # dup half's last column is never produced by the shift copy;
            # zero it once so solo-tap matmuls (bottom weights zero) can't
            # see NaN*0.
            nc.gpsimd.memset(x_sb[C:128, :, WPAD - 1 : WPAD], 0.0)
            for (a, b) in ROW_CHUNKS:
                nc.sync.dma_start(out=x_sb[0:C, a:b, :], in_=x_d[:, a:b, :])
                nc.vector.tensor_copy(
                    x_sb[C:128, a:b, 0 : WPAD - 1],
                    x_sb[0:C, a:b, 1:WPAD],
                )

        for blk in range(NBLK):
            for s in range(2):
                ps = psum.tile([128, NFREE], F32)
                rA = blk * RB + s * ROWS_PER_TILE          # A half: rows 0:8
                rB = blk * RB + 8 + s * ROWS_PER_TILE      # B half: rows 8:16
                for wi in range(6):
                    i = wi % 3                 # tap row
                    j = 0 if wi < 3 else 2     # pairs at col 0, solos at col 2
                    for col, r0 in ((0, rA), (1, rB)):
                        rhs = x_sb[:, r0 + i : r0 + i + ROWS_PER_TILE, j : j + W]
                        nc.tensor.matmul(
                            ps[64 * col : 64 * col + OC, :],
                            lhsT=w_sb[:, wi * OC : (wi + 1) * OC],
                            rhs=rhs,
                            start=(wi == 0),
                            stop=(wi == 5),
                            tile_position=(0, 64 * col),
                            skip_group_check=True,
                        )
                # evict A (partition base 0) on Vector, B on Scalar -- both
                # write bf16 with the bias folded in.
                nc.vector.tensor_scalar_add(
                    o_sb[:, rA * W : (rA + ROWS_PER_TILE) * W],
                    ps[0:OC, :],
                    b_sb[:, 0:1],
                )
                nc.scalar.activation(
                    out=o_sb[:, rB * W : (rB + ROWS_PER_TILE) * W],
                    in_=ps[64:128, :],
                    func=mybir.ActivationFunctionType.Identity,
                    bias=b_sb[:, 0:1],
                    scale=1.0,
                )
            if blk % 2 == 1:
                r0 = (blk - 1) * RB
                nc.sync.dma_start(
                    out=o_d[:, r0 : r0 + 2 * RB, :],
                    in_=o_sb[:, r0 * W : (r0 + 2 * RB) * W].rearrange(
                        "p (r w) -> p r w", r=2 * RB
                    ),
                )
    if split_waits:
        _split_sync_waits(nc)
    return nc


BF16 = mybir.dt.bfloat16

# DMA chunks (padded-row ranges) and finer copy ranges with the block
# index before which each copy triplet is emitted.
DMA_CHUNKS = [(0, 12), (12, 23), (23, 34), (34, 66), (66, 98), (98, 130)]
# per-block copy emission schedule: ("cs"|"t", row0, row1); T copies
# trail their colshift source by roughly a block
# blocks 0-2 run a 6-matmul tap schedule with no T-tile dependency, so
# T copies start at row 46 (first needed by block 3) and Vector stays
# light while the input DMA is still streaming.
HYBRID_BLKS = int(os.environ.get("BASS_HYBRID_BLKS", "0"))
if HYBRID_BLKS == 0:
    EMIT_SEQ = {
        0: [("cs", 0, 12), ("t", 0, 12), ("cs", 12, 23), ("t", 12, 23),
            ("cs", 23, 34), ("t", 23, 34)],
        1: [("cs", 34, 66), ("t", 34, 66)],
        3: [("cs", 66, 98), ("t", 66, 98)],
        5: [("cs", 98, 130), ("t", 98, 130)],
    }
else:
    EMIT_SEQ = {
        0: [("cs", 0, 12), ("cs", 12, 23), ("cs", 23, 34)],
        1: [("cs", 34, 66), ("t", 46, 66)],
        3: [("cs", 66, 98), ("t", 66, 98)],
        5: [("cs", 98, 130), ("t", 98, 130)],
    }
# output DMAs: (emit_after_block, col0, col1) over the split o_sb layout
# (partition 0:64 = channel c, A-rows; 64:128 = channel c, B-rows;
#  col = blk*1024 + row_in_half*128 + w)
OUT_DMAS = [(1, 0, 2048), (3, 2048, 4096), (5, 4096, 6144),
            (6, 6144, 7168)]


def _build_conv_nc_v3():
    NW = 5 * OC + 1
    nc = bass.Bass()
    x_d = nc.declare_dram_parameter("x", [C, HPAD, WPAD], BF16, isOutput=False)
    w_d = nc.declare_dram_parameter("wts", [64, 2 * NW], BF16, isOutput=False)
    o_d = nc.declare_dram_parameter("out", [128, H * W // 2], BF16, isOutput=True)

    with ExitStack() as ctx:
        tc = ctx.enter_context(tile.TileContext(nc))
        singles = ctx.enter_context(tc.tile_pool(name="singles", bufs=1))
        psum = ctx.enter_context(tc.tile_pool(name="psum", bufs=4, space="PSUM"))

        x_sb = singles.tile([128, HPAD, WPAD], BF16)
        t_sb = singles.tile([128, HPAD, WPAD], BF16)
        w_sb = singles.tile([128, 5 * OC + 1], BF16)
        wtmp = singles.tile([64, 2 * NW], BF16)
        o_sb = singles.tile([128, H * W // 2], BF16)
        b32 = singles.tile([128, 1], F32)
        b_ap = b32[:, 0:1]

        def emit_chunk_dma(ci):
            a, b = DMA_CHUNKS[ci]
            nc.sync.dma_start(out=x_sb[0:C, a:b, :], in_=x_d[:, a:b, :])

        def emit_cs(a, b):
            # colshift: x_sb bottom half = x shifted one column left
            nc.vector.tensor_copy(
                x_sb[C:128, a:b, 0 : WPAD - 1], x_sb[0:C, a:b, 1:WPAD]
            )

        def emit_t(a, b):
            # T top = colshift (cols 1:129 only are ever read)
            nc.vector.tensor_copy(
                t_sb[0:C, a:b, 1 : WPAD - 1], x_sb[C:128, a:b, 1 : WPAD - 1]
            )
            # T bottom[r] = colshift[r+1]
            a2 = max(a - 1, 0)
            nc.vector.tensor_copy(
                t_sb[C:128, a2 : b - 1, 1 : WPAD - 1],
                x_sb[C:128, a2 + 1 : b, 1 : WPAD - 1],
            )

        with tc.high_priority(offset=60):
            # dup half's last col is read by the solo tap (zero weights);
            # zero it so NaN*0 can't occur.
            nc.gpsimd.memset(x_sb[C:128, :, WPAD - 1 : WPAD], 0.0)
            emit_chunk_dma(0)
            # weights land folded to 64 partitions (64 descriptors keeps
            # the input chunks flowing); GpSimd unfolds the top half and
            # Vector the bottom (0->64) in parallel.
            nc.sync.dma_start(out=wtmp[:, :], in_=w_d[:, :])
            nc.gpsimd.tensor_copy(w_sb[0:64, :], wtmp[:, 0:NW])
            nc.vector.tensor_copy(w_sb[64:128, :], wtmp[:, NW : 2 * NW])
            nc.vector.tensor_copy(b32[:, 0:1], w_sb[:, 5 * OC : 5 * OC + 1])
            for ci in range(1, len(DMA_CHUNKS)):
                emit_chunk_dma(ci)

        for blk in range(NBLK):
            with tc.high_priority(offset=60):
                for kind, a, b in EMIT_SEQ.get(blk, []):
                    (emit_cs if kind == "cs" else emit_t)(a, b)

            ps = psum.tile([128, 1024], F32)
            # (wi, tile, tap_row, col_off): blocks < HYBRID_BLKS replace the
            # T-pair (wi 3) with two plain solos (wi 5 = k2, wi 6 = k5)
            if blk < HYBRID_BLKS:
                seq = [(0, "x", 0, 0), (1, "x", 1, 0), (2, "x", 2, 0),
                       (4, "x", 2, 2), (5, "x", 0, 2), (6, "x", 1, 2)]
            else:
                seq = [(0, "x", 0, 0), (1, "x", 1, 0), (2, "x", 2, 0),
                       (4, "x", 2, 2), (3, "t", 0, 1)]
            last_wi = seq[-1][0]
            for s in range(2):
                rA = blk * RB + s * 4
                rB = blk * RB + 8 + s * 4
                for wi, tl, ti, tj in seq:
                    for col, r0 in ((0, rA), (1, rB)):
                        srct = x_sb if tl == "x" else t_sb
                        rhs = srct[:, r0 + ti : r0 + ti + 4, tj : tj + W]
                        nc.tensor.matmul(
                            ps[64 * col : 64 * col + OC, s * 512 : (s + 1) * 512],
                            lhsT=w_sb[:, wi * OC : (wi + 1) * OC],
                            rhs=rhs,
                            start=(wi == 0),
                            stop=(wi == last_wi),
                            tile_position=(0, 64 * col),
                            skip_group_check=True,
                        )
            # one 128-lane eviction per block: partitions 0:64 carry the
            # A-half rows, 64:128 the B-half rows (o_sb is stored split;
            # the host unscrambles).  Last block evicts per-subtile so
            # only a [128,512] op trails the final matmul.
            c0 = blk * 1024
            if blk == NBLK - 1:
                # s0 on Scalar (overlaps s1 matmuls), s1 on the idle Vector;
                # each followed by its own half-block output DMA.
                nc.scalar.activation(
                    out=o_sb[:, c0 : c0 + 512],
                    in_=ps[:, 0:512],
                    func=mybir.ActivationFunctionType.Identity,
                    bias=b_ap,
                    scale=1.0,
                )
                nc.sync.dma_start(
                    out=o_d[:, c0 : c0 + 512], in_=o_sb[:, c0 : c0 + 512]
                )
                nc.vector.tensor_scalar_add(
                    o_sb[:, c0 + 512 : c0 + 1024], ps[:, 512:1024], b_ap
                )
                nc.sync.dma_start(
                    out=o_d[:, c0 + 512 : c0 + 1024],
                    in_=o_sb[:, c0 + 512 : c0 + 1024],
                )
            else:
                nc.scalar.activation(
                    out=o_sb[:, c0 : c0 + 1024],
                    in_=ps[:, 0:1024],
                    func=mybir.ActivationFunctionType.Identity,
                    bias=b_ap,
                    scale=1.0,
                )
            for (eb, d0, d1) in OUT_DMAS:
                if eb == blk:
                    nc.sync.dma_start(out=o_d[:, d0:d1], in_=o_sb[:, d0:d1])
    _split_sync_waits(nc)
    return nc


def _pack_dup5(wts, wb):
    """wts [C, 9*OC] tap-major -> [128, 7*OC+1] lhsT layout.

    Column blocks 0-3: K=128 pairs (taps (0,1),(3,4),(6,7),(2,5) with the
    partner tap on the bottom partitions); block 4: solo k8; blocks 5,6:
    solo k2, k5 (hybrid early-block schedule); last col: bias (both
    halves)."""
    w5 = np.zeros((128, 5 * OC + 1), np.float32)
    pairs = [(0, 1), (3, 4), (6, 7), (2, 5)]
    for m, (t, bt) in enumerate(pairs):
        w5[0:C, m * OC : (m + 1) * OC] = wts[:, t * OC : (t + 1) * OC]
        w5[C:128, m * OC : (m + 1) * OC] = wts[:, bt * OC : (bt + 1) * OC]
    w5[0:C, 4 * OC : 5 * OC] = wts[:, 8 * OC : 9 * OC]
    w5[0:OC, 5 * OC] = wb[:, 0]
    w5[OC:128, 5 * OC] = wb[:, 0]
    return np.concatenate([w5[0:64], w5[64:128]], axis=1)


def _unscramble_out(arr):
    """[128, 8192] split layout -> [64, 128, 128]."""
    a = arr[0:64].reshape(OC, NBLK, 8, W)
    b = arr[64:128].reshape(OC, NBLK, 8, W)
    return np.stack([a, b], axis=2).reshape(OC, H, W)


def _host_conv_weights(posi_map, w1, b1, w2, b2, w_ca, bias):
    """Collapse the constant-posi_map weight generator on the host."""
    pm = np.asarray(posi_map, np.float64)[0]              # [16, GP, GP]
    vvec = pm.reshape(POSI_CH, -1)[:, 0]                  # per-channel constant
    h = np.maximum(np.asarray(w1, np.float64) @ vvec + np.asarray(b1, np.float64), 0.0)
    wvec = np.asarray(w2, np.float64) @ h + np.asarray(b2, np.float64)   # [576]
    wk = wvec.reshape(OC, 9)                              # [o, k]
    wca = np.asarray(w_ca, np.float64)                    # [o, c]
    wfull = wk[:, None, :] * wca[:, :, None]              # [o, c, k]
    wts = np.ascontiguousarray(
        wfull.transpose(1, 2, 0).reshape(C, 9 * OC).astype(np.float32)
    )                                                     # [c, k*OC + o]
    wb = np.ascontiguousarray(
        np.asarray(bias, np.float32).reshape(OC, 1)
    )
    return wts, wb


def _pack_dup3(wts):
    """Repack [C, 9*OC] tap-major lhsT into the dup3 layout [128, 6*OC].

    Columns 0:3*OC are K=128 pairs (taps (3p, 3p+1) stacked on the
    partition axis, matching the +1-column-shifted input duplicate);
    columns 3*OC:6*OC are the K=64 solo taps (3q+2), bottom half zero.
    """
    w3 = np.zeros((128, 6 * OC), np.float32)
    for p in range(3):
        w3[0:C, p * OC:(p + 1) * OC] = wts[:, (3 * p) * OC:(3 * p + 1) * OC]
        w3[C:2 * C, p * OC:(p + 1) * OC] = wts[:, (3 * p + 1) * OC:(3 * p + 2) * OC]
        w3[0:C, (3 + p) * OC:(4 + p) * OC] = wts[:, (3 * p + 2) * OC:(3 * p + 3) * OC]
    return w3


def _numpy_reference(x, offset, posi_map, w1, b1, w2, b2, w_ca, bias):
    """Exact numpy port of reference.py (general-input fallback)."""
    x = np.asarray(x, np.float32)
    offset = np.asarray(offset, np.float32)
    posi_map = np.asarray(posi_map, np.float32)
    w1 = np.asarray(w1, np.float32)
    b1 = np.asarray(b1, np.float32)
    w2 = np.asarray(w2, np.float32)
    b2 = np.asarray(b2, np.float32)
    w_ca = np.asarray(w_ca, np.float32)
    bias = np.asarray(bias, np.float32)

    Bq, _, Hq, Wq = x.shape
    dx = offset[:, 0] * (2.0 / max(Wq - 1, 1)) * 0.5
    dy = offset[:, 1] * (2.0 / max(Hq - 1, 1)) * 0.5
    ys = np.linspace(-1.0, 1.0, Hq, dtype=x.dtype)
    xs = np.linspace(-1.0, 1.0, Wq, dtype=x.dtype)
    gx = xs[None, None, :] + dx
    gy = ys[None, :, None] + dy
    img = np.broadcast_to(posi_map, (Bq, posi_map.shape[1], GP, GP))

    Hp = Wp = GP
    imgT = img.transpose(0, 2, 3, 1)                      # [B, Hp, Wp, C]
    ix = np.clip((gx + 1.0) * 0.5 * (Wp - 1), 0.0, Wp - 1)
    iy = np.clip((gy + 1.0) * 0.5 * (Hp - 1), 0.0, Hp - 1)
    x0 = np.floor(ix).astype(np.int32)
    y0 = np.floor(iy).astype(np.int32)
    x1 = np.minimum(x0 + 1, Wp - 1)
    y1 = np.minimum(y0 + 1, Hp - 1)
    wx = (ix - x0.astype(ix.dtype))[..., None]
    wy = (iy - y0.astype(iy.dtype))[..., None]
    bb = np.arange(Bq)[:, None, None]
    v00 = imgT[bb, y0, x0]
    v01 = imgT[bb, y0, x1]
    v10 = imgT[bb, y1, x0]
    v11 = imgT[bb, y1, x1]
    top = v00 * (1 - wx) + v01 * wx
    bot = v10 * (1 - wx) + v11 * wx
    up = (top * (1 - wy) + bot * wy).transpose(0, 3, 1, 2)  # [B, 16, H, W]

    h = np.maximum(np.einsum('oc,bchw->bohw', w1, up) + b1[None, :, None, None], 0.0)
    weights = np.einsum('oc,bchw->bohw', w2, h) + b2[None, :, None, None]
    weights = weights.reshape(Bq, OC, KK * KK, Hq, Wq)
    x_adapt = np.einsum('oc,bchw->bohw', w_ca, x)
    xp = np.pad(x_adapt, ((0, 0), (0, 0), (1, 1), (1, 1)))
    patches = np.stack(
        [xp[:, :, i:i + Hq, j:j + Wq] for i in range(KK) for j in range(KK)],
        axis=2,
    )
    out = (weights * patches).sum(axis=2) + bias
    return out.astype(np.float32)


def kernel(**inputs):
    global _cached_nc, last_results
    x = np.ascontiguousarray(np.asarray(inputs["x"], np.float32))
    posi_map = np.asarray(inputs["posi_map"], np.float32)

    per_ch = posi_map.reshape(posi_map.shape[0] * posi_map.shape[1], -1)
    if not np.all(per_ch == per_ch[:, :1]):
        # general (spatially varying posi_map) fallback: exact numpy port
        return _numpy_reference(**{k: inputs[k] for k in (
            "x", "offset", "posi_map", "w1", "b1", "w2", "b2", "w_ca", "bias")})

    wts, wb = _host_conv_weights(
        posi_map, inputs["w1"], inputs["b1"], inputs["w2"], inputs["b2"],
        inputs["w_ca"], inputs["bias"],
    )

    variant = os.environ.get("BASS_KERNEL_VARIANT", "dual_v3")
    global _cached_variant
    if _cached_nc is None or _cached_variant != variant:
        if variant == "dual_v3":
            _cached_nc = _build_conv_nc_v3()
        elif variant == "sbuf_v2":
            _cached_nc = _build_conv_nc_v2()
        else:
            _cached_nc = _build_conv_nc(variant)
        _cached_variant = variant

    xpad = np.pad(x, ((0, 0), (0, 0), (1, 1), (1, 1)))
    base_variant = variant[:-4] if variant.endswith("_o16") else variant
    if base_variant in ("bf16_dup3", "sbuf_v2"):
        wts = _pack_dup3(wts)
    elif base_variant == "dual_v3":
        wts = _pack_dup5(wts, wb)
    if base_variant in ("bf16_tiled", "bf16_dup3", "sbuf_v2", "dual_v3"):
        import ml_dtypes

        xpad = xpad.astype(ml_dtypes.bfloat16)
        wts = wts.astype(ml_dtypes.bfloat16)
    if base_variant == "dual_v3":
        in_maps = [{"x": xpad[i], "wts": wts} for i in range(NCORES)]
    else:
        in_maps = [{"x": xpad[i], "wts": wts, "wb": wb} for i in range(NCORES)]
    trace = os.environ.get("BASS_KERNEL_TRACE", "0") == "1"
    if trace:
        _ensure_ntff_hook()
    res = run_bass_kernel_spmd(
        _cached_nc, in_maps, list(range(NCORES)), trace=trace
    )
    last_results = res
    if base_variant == "dual_v3":
        out = np.stack(
            [
                _unscramble_out(np.asarray(res.results[i]["out"], np.float32))
                for i in range(NCORES)
            ],
            axis=0,
        )
    else:
        out = np.stack(
            [np.asarray(res.results[i]["out"], np.float32) for i in range(NCORES)],
            axis=0,
        )
    return out



# revision 20
# speedup vs baseline: 1.0250x; 1.0250x over previous
"""Trainium2 Bass kernel for nn_CPADConvOffsetStage.

The reference module is:
  up_posi = grid_sample_bilinear_border(posi_map -> [B,16,GP,GP], grid = base + offset*scale)
  h       = relu(w1 @ up_posi + b1)           (1x1 conv)
  weights = (w2 @ h + b2).reshape(B,64,9,H,W) (1x1 conv -> per-pixel 3x3 kernels)
  x_adapt = w_ca @ x                          (1x1 conv)
  out     = sum_k weights[:,:,k] * unfold3x3(x_adapt)[:,:,k] + bias

In setup_inputs() posi_map is spatially constant per channel (jnp.ones).
Bilinear interpolation weights sum to exactly 1, so up_posi is spatially
constant => h, weights are spatially constant => the whole module reduces
to ONE dense 3x3 convolution with host-precomputable weights
    Wfull[o,c,k] = wk[o,k] * w_ca[o,c],   wk = (w2 @ relu(w1 @ v + b1) + b2)
plus the bias.  The kernel below runs that conv data-parallel over batch
(1 batch image per NeuronCore, 8 cores).

If posi_map is NOT per-channel spatially constant (never the case for the
shipped setup_inputs), we fall back to an exact numpy port of the
reference.
"""

import os
import numpy as np
from contextlib import ExitStack

import concourse.bass as bass
import concourse.tile as tile
from concourse import mybir
from concourse.bass_utils import run_bass_kernel_spmd

# Problem constants (hardcoded per contract)
B, C, H, W = 8, 64, 128, 128
OC = 64
KK = 3
POSI_CH, GP = 16, 16
NCORES = 8
F32 = mybir.dt.float32

HPAD, WPAD = H + 2, W + 2      # host-padded image (130 x 130)
ROWS_PER_TILE = 4              # 4 rows * 128 cols = 512 = max fp32 moving free dim
NFREE = ROWS_PER_TILE * W      # 512
RB = 16                        # output rows per SBUF block
NBLK = H // RB                 # blocks per image
SUB = RB // ROWS_PER_TILE      # psum tiles per block
NXB = 4                        # rotated input block buffers
F32R = mybir.dt.float32r       # fp32 storage, single-pass relaxed-precision matmul

_cached_nc = None
_cached_variant = None
last_results = None            # test harness introspection


def _ensure_ntff_hook():
    """Register the axon NTFF-profile hook that this image's antenv lacks.

    run_bass_kernel_spmd(trace=True) under axon needs
    antenv.axon_hooks.get_axon_ntff_profile_hook; the hook machinery
    exists in trn_agent_boot but was never registered because
    antenv.axon_hooks is missing.  Recreate the module in sys.modules.
    """
    import sys
    import types

    if "antenv.axon_hooks" in sys.modules:
        return
    try:
        from trn_agent_boot.trn_boot import _ntff_profile_via_ctypes

        hook = _ntff_profile_via_ctypes("/opt/axon/libaxon_pjrt.so")
    except Exception:
        hook = None
    mod = types.ModuleType("antenv.axon_hooks")
    mod.get_axon_ntff_profile_hook = lambda: hook
    mod.set_axon_ntff_profile_hook = lambda h: None
    sys.modules["antenv.axon_hooks"] = mod
    try:
        import antenv

        antenv.axon_hooks = mod
    except Exception:
        pass


def _build_conv_nc(variant="bf16_tiled", split_waits=True):
    """3x3 conv, 64->64 ch, on one (host-padded) [64,130,130] image.

    SPMD over 8 cores, one batch image per core.  The input arrives
    zero-padded from the host so every 3x3 tap is a plain shifted
    window read; no memsets / halo special cases on device (this also
    keeps every Matmult at <=2 semaphore waits — walrus rejects 3+ on
    the LDWEIGHTS struct).
    """
    o16 = variant.endswith("_o16")
    base_variant = variant[:-4] if o16 else variant
    tiled = base_variant in ("bf16_tiled", "bf16_dup3")
    dup = base_variant == "bf16_dup3"
    mm_dt = mybir.dt.bfloat16 if tiled else F32R
    o_dt = mybir.dt.bfloat16 if o16 else F32
    xb_parts = 128 if dup else C
    w_cols = 6 * OC if dup else 9 * OC
    nc = bass.Bass()
    x_d = nc.declare_dram_parameter("x", [C, HPAD, WPAD], mm_dt, isOutput=False)
    w_d = nc.declare_dram_parameter("wts", [xb_parts, w_cols], mm_dt, isOutput=False)
    b_d = nc.declare_dram_parameter("wb", [OC, 1], F32, isOutput=False)
    o_d = nc.declare_dram_parameter("out", [OC, H, W], o_dt, isOutput=True)

    with ExitStack() as ctx:
        tc = ctx.enter_context(tile.TileContext(nc))
        singles = ctx.enter_context(tc.tile_pool(name="singles", bufs=1))
        outs = ctx.enter_context(tc.tile_pool(name="outs", bufs=4))
        psum = ctx.enter_context(
            tc.tile_pool(name="psum", bufs=8 if dup else 4, space="PSUM")
        )
        tmps = ctx.enter_context(tc.tile_pool(name="tmps", bufs=4))

        w_sb = singles.tile([xb_parts, w_cols], mm_dt)
        nc.sync.dma_start(out=w_sb[:, :], in_=w_d[:, :])
        b_sb = singles.tile([OC, 1], F32)
        nc.sync.dma_start(out=b_sb[:, :], in_=b_d[:, :])

        # Rotated input blocks: 18 padded rows each (16 output rows + halo)
        xbs = []
        for i in range(NXB):
            xb_buf = singles.tile([xb_parts, RB + 2, WPAD], mm_dt, tag=f"xb{i}", name=f"xb{i}")
            if dup:
                # The duplicate (shifted) half never receives its last
                # column from DMA; zero it once so K=128 solo matmuls
                # (whose bottom-half weights are zero) can't hit NaN*0.
                nc.vector.memset(xb_buf[C : 2 * C, :, WPAD - 1 : WPAD], 0.0)
            xbs.append(xb_buf)

        # Tap k -> column group: A (psum partitions 0:64, tile_position (0,0))
        # gets taps 0,2,4,6,8; B (64:128, (0,64)) gets 1,3,5,7.  The two
        # column groups of the 128x128 PE array run concurrently (separate
        # XBUS streams), so 9 taps cost ~5 matmul slots instead of 9.
        for blk in range(NBLK):
            xb = xbs[blk % NXB]
            # Prefetch the input pipeline aggressively: schedule each block's
            # load + shifted-duplicate copy as if issued a block earlier, so
            # the first matmul of a block never waits on them (the profile
            # showed ~2.5us PE gaps at early block boundaries).
            with tc.high_priority(offset=60):
                nc.sync.dma_start(
                    out=xb[0:C, :, :],
                    in_=x_d[:, blk * RB : blk * RB + RB + 2, :],
                )
                if dup:
                    # partitions 64:128 = same rows shifted one column left,
                    # so a K=128 matmul contracts two horizontally-adjacent
                    # taps at once.  Derived on-chip (cross-partition copy)
                    # instead of a second DRAM read — DMA is the bottleneck.
                    nc.vector.tensor_copy(
                        xb[C : 2 * C, :, 0 : WPAD - 1],
                        xb[0:C, :, 1:WPAD],
                    )
            o_blk = outs.tile([OC, RB * W], o_dt)
            for s in range(SUB):
                r0 = s * ROWS_PER_TILE        # row offset within block
                if dup:
                    ps = psum.tile([128, NFREE], F32)
                    # 6 all-K=128 matmuls -> 3 col-tiled slots (K=128 streams
                    # ~2x faster per column than K=64 on this part, and the
                    # solo taps' bottom-half weights are zero):
                    #  A: pair(0,1) | pair(6,7) | solo k5
                    #  B: pair(3,4) | solo k2   | solo k8
                    mms = [
                        (0, 0, 0, True, False),   # pair row 0 -> A
                        (1, 1, 1, True, False),   # pair row 1 -> B
                        (2, 2, 0, False, False),  # pair row 2 -> A
                        (3, 0, 1, False, False),  # k2 (row 0, col 2) -> B
                        (4, 1, 0, False, True),   # k5 (row 1, col 2) -> A
                        (5, 2, 1, False, True),   # k8 (row 2, col 2) -> B
                    ]
                    for wi, i, col, st, sp in mms:
                        j = 0 if wi < 3 else 2
                        rhs = xb[:, r0 + i : r0 + i + ROWS_PER_TILE, j : j + W]
                        nc.tensor.matmul(
                            ps[64 * col : 64 * col + OC, :],
                            lhsT=w_sb[:, wi * OC : (wi + 1) * OC],
                            rhs=rhs,
                            start=st,
                            stop=sp,
                            tile_position=(0, 64 * col),
                            skip_group_check=True,
                        )
                elif tiled:
                    ps = psum.tile([128, NFREE], F32)
                    for k in range(9):
                        i, j = divmod(k, 3)
                        rhs = xb[:, r0 + i : r0 + i + ROWS_PER_TILE, j : j + W]
                        col = k % 2           # even taps -> A, odd -> B
                        nc.tensor.matmul(
                            ps[64 * col : 64 * col + OC, :],
                            lhsT=w_sb[:, k * OC : (k + 1) * OC],
                            rhs=rhs,
                            start=(k < 2),
                            stop=(k >= 7),
                            tile_position=(0, 64 * col),
                            skip_group_check=True,
                        )
                if tiled:
                    # Fold: B half evicted by ACT (with bias), A half added
                    # by DVE (walrus allows only one PSUM input per DVE op).
                    ps_a = ps[0:OC, 0:NFREE]
                    ps_b = ps[64:128, 0:NFREE]
                    tmpb = tmps.tile([OC, NFREE], F32)
                    nc.scalar.activation(
                        out=tmpb[:, :],
                        in_=ps_b,
                        func=mybir.ActivationFunctionType.Identity,
                        bias=b_sb[:, 0:1],
                        scale=1.0,
                    )
                    nc.vector.tensor_add(
                        o_blk[:, s * NFREE : (s + 1) * NFREE],
                        ps_a,
                        tmpb[:, :],
                    )
                else:
                    ps = psum.tile([OC, NFREE], F32)
                    for k in range(9):
                        i, j = divmod(k, 3)
                        rhs = xb[:, r0 + i : r0 + i + ROWS_PER_TILE, j : j + W]
                        nc.tensor.matmul(
                            ps[:, :],
                            lhsT=w_sb[:, k * OC : (k + 1) * OC],
                            rhs=rhs,
                            start=(k == 0),
                            stop=(k == 8),
                        )
                    nc.scalar.activation(
                        out=o_blk[:, s * NFREE : (s + 1) * NFREE],
                        in_=ps[:, :],
                        func=mybir.ActivationFunctionType.Identity,
                        bias=b_sb[:, 0:1],
                        scale=1.0,
                    )
            # Two half-block output DMAs: the first half drains while the
            # second half is still being computed (shorter kernel tail).
            HRB = RB // 2
            for h in range(2):
                nc.sync.dma_start(
                    out=o_d[:, blk * RB + h * HRB : blk * RB + (h + 1) * HRB, :],
                    in_=o_blk[:, h * HRB * W : (h + 1) * HRB * W].rearrange(
                        "p (r w) -> p r w", r=HRB
                    ),
                )
    if split_waits:
        _split_sync_waits(nc)
    return nc


def _split_sync_waits(nc, limit=1):
    """Hoist extra sync waits onto injected wait-only EventSemaphore ops.

    The neuronxcc walrus used under axon rejects compute instructions
    carrying more than one sync wait ("Too many sync wait commands", e.g.
    S3_LW / S3D3_AC structs).  Tile's sem assignment emits up to ~3.
    For every instruction with >limit waits, keep the first `limit` and
    prepend one wait-only EventSemaphore per extra wait on the same
    engine (same program position => same semantics).
    """
    import copy as _copy

    f = nc.m.functions[0]
    template = None
    for blk in f.blocks:
        for inst in blk.instructions:
            if type(inst).__name__ == "InstEventSemaphore":
                template = inst
                break
        if template is not None:
            break
    if template is None:
        return
    n_split = 0
    for blk in f.blocks:
        new_list = []
        changed = False
        for inst in blk.instructions:
            si = getattr(inst, "sync_info", None)
            ilimit = limit
            waits = list(si.on_wait) if (si and si.on_wait) else []
            if len(waits) > ilimit:
                for w in waits[ilimit:]:
                    ev = _copy.deepcopy(template)
                    ev.name = f"waitsplit_{n_split}"
                    n_split += 1
                    ev.engine = inst.engine
                    ev.sync_info = mybir.SyncInfo(on_wait=[w], on_update=[])
                    new_list.append(ev)
                inst.sync_info = mybir.SyncInfo(
                    on_wait=waits[:ilimit], on_update=list(si.on_update or [])
                )
                changed = True
            new_list.append(inst)
        if changed:
            blk.instructions = new_list


ROW_CHUNKS = [(0, 18), (18, 50), (50, 82), (82, 114), (114, 130)]


def _build_conv_nc_v2(split_waits=True):
    """SBUF-resident 3x3 conv, 64->64 ch, one padded [64,130,130] image/core.

    Differences vs _build_conv_nc(bf16_dup3):
      * The whole padded input lives in one SBUF tile (partitions 0:64 =
        x, 64:128 = one-column-left-shifted dup), loaded by 5 chunked
        DMAs + per-chunk shift copies -- no per-block halo re-reads and
        the first matmul only waits on an 18-row chunk.
      * The two PE column groups compute DISJOINT row ranges (A = rows
        0:8 of each 16-row block, B = rows 8:16) instead of partial
        sums of the same pixels, so the per-subtile PSUM fold
        (Scalar IDENTITY + Vector ADD) disappears entirely; each PSUM
        half is evicted once, with the bias, directly to bf16.
      * Evictions alternate Vector (A half, tensor_scalar_add) and
        Scalar (B half, activation+bias) so neither engine is a
        serial bottleneck.
      * Output is a single SBUF-resident bf16 image written back by 4
        large DMAs (one per 2 blocks) -- ~4x fewer, larger output
        descriptors than the fp32 half-block DMAs.
    """
    nc = bass.Bass()
    x_d = nc.declare_dram_parameter("x", [C, HPAD, WPAD], mybir.dt.bfloat16,
                                    isOutput=False)
    w_d = nc.declare_dram_parameter("wts", [128, 6 * OC], mybir.dt.bfloat16,
                                    isOutput=False)
    b_d = nc.declare_dram_parameter("wb", [OC, 1], F32, isOutput=False)
    o_d = nc.declare_dram_parameter("out", [OC, H, W], mybir.dt.bfloat16,
                                    isOutput=True)

    with ExitStack() as ctx:
        tc = ctx.enter_context(tile.TileContext(nc))
        singles = ctx.enter_context(tc.tile_pool(name="singles", bufs=1))
        psum = ctx.enter_context(tc.tile_pool(name="psum", bufs=8, space="PSUM"))

        w_sb = singles.tile([128, 6 * OC], mybir.dt.bfloat16)
        b_sb = singles.tile([OC, 1], F32)
        x_sb = singles.tile([128, HPAD, WPAD], mybir.dt.bfloat16)
        o_sb = singles.tile([OC, H * W], mybir.dt.bfloat16)

        with tc.high_priority(offset=60):
            nc.sync.dma_start(out=w_sb[:, :], in_=w_d[:, :])
            nc.sync.dma_start(out=b_sb[:, :], in_=b_d[:, :])
            # dup half's last column is never produced by the shift copy;
            # zero it once so solo-tap matmuls (bottom weights zero) can't
            # see NaN*0.
            nc.gpsimd.memset(x_sb[C:128, :, WPAD - 1 : WPAD], 0.0)
            for (a, b) in ROW_CHUNKS:
                nc.sync.dma_start(out=x_sb[0:C, a:b, :], in_=x_d[:, a:b, :])
                nc.vector.tensor_copy(
                    x_sb[C:128, a:b, 0 : WPAD - 1],
                    x_sb[0:C, a:b, 1:WPAD],
                )

        for blk in range(NBLK):
            for s in range(2):
                ps = psum.tile([128, NFREE], F32)
                rA = blk * RB + s * ROWS_PER_TILE          # A half: rows 0:8
                rB = blk * RB + 8 + s * ROWS_PER_TILE      # B half: rows 8:16
                for wi in range(6):
                    i = wi % 3                 # tap row
                    j = 0 if wi < 3 else 2     # pairs at col 0, solos at col 2
                    for col, r0 in ((0, rA), (1, rB)):
                        rhs = x_sb[:, r0 + i : r0 + i + ROWS_PER_TILE, j : j + W]
                        nc.tensor.matmul(
                            ps[64 * col : 64 * col + OC, :],
                            lhsT=w_sb[:, wi * OC : (wi + 1) * OC],
                            rhs=rhs,
                            start=(wi == 0),
                            stop=(wi == 5),
                            tile_position=(0, 64 * col),
                            skip_group_check=True,
                        )
                # evict A (partition base 0) on Vector, B on Scalar -- both
                # write bf16 with the bias folded in.
                nc.vector.tensor_scalar_add(
                    o_sb[:, rA * W : (rA + ROWS_PER_TILE) * W],
                    ps[0:OC, :],
                    b_sb[:, 0:1],
                )
                nc.scalar.activation(
                    out=o_sb[:, rB * W : (rB + ROWS_PER_TILE) * W],
                    in_=ps[64:128, :],
                    func=mybir.ActivationFunctionType.Identity,
                    bias=b_sb[:, 0:1],
                    scale=1.0,
                )
            if blk % 2 == 1:
                r0 = (blk - 1) * RB
                nc.sync.dma_start(
                    out=o_d[:, r0 : r0 + 2 * RB, :],
                    in_=o_sb[:, r0 * W : (r0 + 2 * RB) * W].rearrange(
                        "p (r w) -> p r w", r=2 * RB
                    ),
                )
    if split_waits:
        _split_sync_waits(nc)
    return nc


BF16 = mybir.dt.bfloat16

# DMA chunks (padded-row ranges) and finer copy ranges with the block
# index before which each copy triplet is emitted.
DMA_CHUNKS = [(0, 12), (12, 23), (23, 34), (34, 66), (66, 98), (98, 130)]
# per-block copy emission schedule: ("cs"|"t", row0, row1); T copies
# trail their colshift source by roughly a block
# blocks 0-2 run a 6-matmul tap schedule with no T-tile dependency, so
# T copies start at row 46 (first needed by block 3) and Vector stays
# light while the input DMA is still streaming.
HYBRID_BLKS = int(os.environ.get("BASS_HYBRID_BLKS", "0"))
if HYBRID_BLKS == 0:
    EMIT_SEQ = {
        0: [("cs", 0, 12), ("t", 0, 12), ("cs", 12, 23), ("t", 12, 23),
            ("cs", 23, 34), ("t", 23, 34)],
        1: [("cs", 34, 66), ("t", 34, 66)],
        3: [("cs", 66, 98), ("t", 66, 98)],
        5: [("cs", 98, 130), ("t", 98, 130)],
    }
else:
    EMIT_SEQ = {
        0: [("cs", 0, 12), ("cs", 12, 23), ("cs", 23, 34)],
        1: [("cs", 34, 66), ("t", 46, 66)],
        3: [("cs", 66, 98), ("t", 66, 98)],
        5: [("cs", 98, 130), ("t", 98, 130)],
    }
# output DMAs: (emit_after_block, col0, col1) over the split o_sb layout
# (partition 0:64 = channel c, A-rows; 64:128 = channel c, B-rows;
#  col = blk*1024 + row_in_half*128 + w)
OUT_DMAS = [(1, 0, 2048), (3, 2048, 4096), (5, 4096, 6144),
            (6, 6144, 7168)]


def _build_conv_nc_v3():
    nc = bass.Bass()
    x_d = nc.declare_dram_parameter("x", [C, HPAD, WPAD], BF16, isOutput=False)
    w_d = nc.declare_dram_parameter("wts", [128, 7 * OC + 1], BF16,
                                    isOutput=False)
    o_d = nc.declare_dram_parameter("out", [128, H * W // 2], BF16, isOutput=True)

    with ExitStack() as ctx:
        tc = ctx.enter_context(tile.TileContext(nc))
        singles = ctx.enter_context(tc.tile_pool(name="singles", bufs=1))
        psum = ctx.enter_context(tc.tile_pool(name="psum", bufs=4, space="PSUM"))

        x_sb = singles.tile([128, HPAD, WPAD], BF16)
        t_sb = singles.tile([128, HPAD, WPAD], BF16)
        w_sb = singles.tile([128, 7 * OC + 1], BF16)
        o_sb = singles.tile([128, H * W // 2], BF16)
        b32 = singles.tile([128, 1], F32)
        b_ap = b32[:, 0:1]

        def emit_chunk_dma(ci):
            a, b = DMA_CHUNKS[ci]
            nc.sync.dma_start(out=x_sb[0:C, a:b, :], in_=x_d[:, a:b, :])

        def emit_cs(a, b):
            # colshift: x_sb bottom half = x shifted one column left
            nc.vector.tensor_copy(
                x_sb[C:128, a:b, 0 : WPAD - 1], x_sb[0:C, a:b, 1:WPAD]
            )

        def emit_t(a, b):
            # T top = colshift (cols 1:129 only are ever read)
            nc.vector.tensor_copy(
                t_sb[0:C, a:b, 1 : WPAD - 1], x_sb[C:128, a:b, 1 : WPAD - 1]
            )
            # T bottom[r] = colshift[r+1]
            a2 = max(a - 1, 0)
            nc.vector.tensor_copy(
                t_sb[C:128, a2 : b - 1, 1 : WPAD - 1],
                x_sb[C:128, a2 + 1 : b, 1 : WPAD - 1],
            )

        with tc.high_priority(offset=60):
            # dup half's last col is read by the solo tap (zero weights);
            # zero it so NaN*0 can't occur.
            nc.gpsimd.memset(x_sb[C:128, :, WPAD - 1 : WPAD], 0.0)
            emit_chunk_dma(0)
            nc.sync.dma_start(out=w_sb[:, :], in_=w_d[:, :])
            nc.vector.tensor_copy(b32[:, 0:1], w_sb[:, 7 * OC : 7 * OC + 1])
            for ci in range(1, len(DMA_CHUNKS)):
                emit_chunk_dma(ci)

        for blk in range(NBLK):
            with tc.high_priority(offset=60):
                for kind, a, b in EMIT_SEQ.get(blk, []):
                    (emit_cs if kind == "cs" else emit_t)(a, b)

            ps = psum.tile([128, 1024], F32)
            # (wi, tile, tap_row, col_off): blocks < HYBRID_BLKS replace the
            # T-pair (wi 3) with two plain solos (wi 5 = k2, wi 6 = k5)
            if blk < HYBRID_BLKS:
                seq = [(0, "x", 0, 0), (1, "x", 1, 0), (2, "x", 2, 0),
                       (4, "x", 2, 2), (5, "x", 0, 2), (6, "x", 1, 2)]
            else:
                seq = [(0, "x", 0, 0), (1, "x", 1, 0), (2, "x", 2, 0),
                       (4, "x", 2, 2), (3, "t", 0, 1)]
            last_wi = seq[-1][0]
            for s in range(2):
                rA = blk * RB + s * 4
                rB = blk * RB + 8 + s * 4
                for wi, tl, ti, tj in seq:
                    for col, r0 in ((0, rA), (1, rB)):
                        srct = x_sb if tl == "x" else t_sb
                        rhs = srct[:, r0 + ti : r0 + ti + 4, tj : tj + W]
                        nc.tensor.matmul(
                            ps[64 * col : 64 * col + OC, s * 512 : (s + 1) * 512],
                            lhsT=w_sb[:, wi * OC : (wi + 1) * OC],
                            rhs=rhs,
                            start=(wi == 0),
                            stop=(wi == last_wi),
                            tile_position=(0, 64 * col),
                            skip_group_check=True,
                        )
            # one 128-lane eviction per block: partitions 0:64 carry the
            # A-half rows, 64:128 the B-half rows (o_sb is stored split;
            # the host unscrambles).  Last block evicts per-subtile so
            # only a [128,512] op trails the final matmul.
            c0 = blk * 1024
            if blk == NBLK - 1:
                # s0 on Scalar (overlaps s1 matmuls), s1 on the idle Vector;
                # each followed by its own half-block output DMA.
                nc.scalar.activation(
                    out=o_sb[:, c0 : c0 + 512],
                    in_=ps[:, 0:512],
                    func=mybir.ActivationFunctionType.Identity,
                    bias=b_ap,
                    scale=1.0,
                )
                nc.sync.dma_start(
                    out=o_d[:, c0 : c0 + 512], in_=o_sb[:, c0 : c0 + 512]
                )
                nc.vector.tensor_scalar_add(
                    o_sb[:, c0 + 512 : c0 + 1024], ps[:, 512:1024], b_ap
                )
                nc.sync.dma_start(
                    out=o_d[:, c0 + 512 : c0 + 1024],
                    in_=o_sb[:, c0 + 512 : c0 + 1024],
                )
            else:
                nc.scalar.activation(
                    out=o_sb[:, c0 : c0 + 1024],
                    in_=ps[:, 0:1024],
                    func=mybir.ActivationFunctionType.Identity,
                    bias=b_ap,
                    scale=1.0,
                )
            for (eb, d0, d1) in OUT_DMAS:
                if eb == blk:
                    nc.sync.dma_start(out=o_d[:, d0:d1], in_=o_sb[:, d0:d1])
    _split_sync_waits(nc)
    return nc


def _pack_dup5(wts, wb):
    """wts [C, 9*OC] tap-major -> [128, 7*OC+1] lhsT layout.

    Column blocks 0-3: K=128 pairs (taps (0,1),(3,4),(6,7),(2,5) with the
    partner tap on the bottom partitions); block 4: solo k8; blocks 5,6:
    solo k2, k5 (hybrid early-block schedule); last col: bias (both
    halves)."""
    w5 = np.zeros((128, 7 * OC + 1), np.float32)
    pairs = [(0, 1), (3, 4), (6, 7), (2, 5)]
    for m, (t, bt) in enumerate(pairs):
        w5[0:C, m * OC : (m + 1) * OC] = wts[:, t * OC : (t + 1) * OC]
        w5[C:128, m * OC : (m + 1) * OC] = wts[:, bt * OC : (bt + 1) * OC]
    for m, t in ((4, 8), (5, 2), (6, 5)):
        w5[0:C, m * OC : (m + 1) * OC] = wts[:, t * OC : (t + 1) * OC]
    w5[0:OC, 7 * OC] = wb[:, 0]
    w5[OC:128, 7 * OC] = wb[:, 0]
    return w5


def _unscramble_out(arr):
    """[128, 8192] split layout -> [64, 128, 128]."""
    a = arr[0:64].reshape(OC, NBLK, 8, W)
    b = arr[64:128].reshape(OC, NBLK, 8, W)
    return np.stack([a, b], axis=2).reshape(OC, H, W)


def _host_conv_weights(posi_map, w1, b1, w2, b2, w_ca, bias):
    """Collapse the constant-posi_map weight generator on the host."""
    pm = np.asarray(posi_map, np.float64)[0]              # [16, GP, GP]
    vvec = pm.reshape(POSI_CH, -1)[:, 0]                  # per-channel constant
    h = np.maximum(np.asarray(w1, np.float64) @ vvec + np.asarray(b1, np.float64), 0.0)
    wvec = np.asarray(w2, np.float64) @ h + np.asarray(b2, np.float64)   # [576]
    wk = wvec.reshape(OC, 9)                              # [o, k]
    wca = np.asarray(w_ca, np.float64)                    # [o, c]
    wfull = wk[:, None, :] * wca[:, :, None]              # [o, c, k]
    wts = np.ascontiguousarray(
        wfull.transpose(1, 2, 0).reshape(C, 9 * OC).astype(np.float32)
    )                                                     # [c, k*OC + o]
    wb = np.ascontiguousarray(
        np.asarray(bias, np.float32).reshape(OC, 1)
    )
    return wts, wb


def _pack_dup3(wts):
    """Repack [C, 9*OC] tap-major lhsT into the dup3 layout [128, 6*OC].

    Columns 0:3*OC are K=128 pairs (taps (3p, 3p+1) stacked on the
    partition axis, matching the +1-column-shifted input duplicate);
    columns 3*OC:6*OC are the K=64 solo taps (3q+2), bottom half zero.
    """
    w3 = np.zeros((128, 6 * OC), np.float32)
    for p in range(3):
        w3[0:C, p * OC:(p + 1) * OC] = wts[:, (3 * p) * OC:(3 * p + 1) * OC]
        w3[C:2 * C, p * OC:(p + 1) * OC] = wts[:, (3 * p + 1) * OC:(3 * p + 2) * OC]
        w3[0:C, (3 + p) * OC:(4 + p) * OC] = wts[:, (3 * p + 2) * OC:(3 * p + 3) * OC]
    return w3


def _numpy_reference(x, offset, posi_map, w1, b1, w2, b2, w_ca, bias):
    """Exact numpy port of reference.py (general-input fallback)."""
    x = np.asarray(x, np.float32)
    offset = np.asarray(offset, np.float32)
    posi_map = np.asarray(posi_map, np.float32)
    w1 = np.asarray(w1, np.float32)
    b1 = np.asarray(b1, np.float32)
    w2 = np.asarray(w2, np.float32)
    b2 = np.asarray(b2, np.float32)
    w_ca = np.asarray(w_ca, np.float32)
    bias = np.asarray(bias, np.float32)

    Bq, _, Hq, Wq = x.shape
    dx = offset[:, 0] * (2.0 / max(Wq - 1, 1)) * 0.5
    dy = offset[:, 1] * (2.0 / max(Hq - 1, 1)) * 0.5
    ys = np.linspace(-1.0, 1.0, Hq, dtype=x.dtype)
    xs = np.linspace(-1.0, 1.0, Wq, dtype=x.dtype)
    gx = xs[None, None, :] + dx
    gy = ys[None, :, None] + dy
    img = np.broadcast_to(posi_map, (Bq, posi_map.shape[1], GP, GP))

    Hp = Wp = GP
    imgT = img.transpose(0, 2, 3, 1)                      # [B, Hp, Wp, C]
    ix = np.clip((gx + 1.0) * 0.5 * (Wp - 1), 0.0, Wp - 1)
    iy = np.clip((gy + 1.0) * 0.5 * (Hp - 1), 0.0, Hp - 1)
    x0 = np.floor(ix).astype(np.int32)
    y0 = np.floor(iy).astype(np.int32)
    x1 = np.minimum(x0 + 1, Wp - 1)
    y1 = np.minimum(y0 + 1, Hp - 1)
    wx = (ix - x0.astype(ix.dtype))[..., None]
    wy = (iy - y0.astype(iy.dtype))[..., None]
    bb = np.arange(Bq)[:, None, None]
    v00 = imgT[bb, y0, x0]
    v01 = imgT[bb, y0, x1]
    v10 = imgT[bb, y1, x0]
    v11 = imgT[bb, y1, x1]
    top = v00 * (1 - wx) + v01 * wx
    bot = v10 * (1 - wx) + v11 * wx
    up = (top * (1 - wy) + bot * wy).transpose(0, 3, 1, 2)  # [B, 16, H, W]

    h = np.maximum(np.einsum('oc,bchw->bohw', w1, up) + b1[None, :, None, None], 0.0)
    weights = np.einsum('oc,bchw->bohw', w2, h) + b2[None, :, None, None]
    weights = weights.reshape(Bq, OC, KK * KK, Hq, Wq)
    x_adapt = np.einsum('oc,bchw->bohw', w_ca, x)
    xp = np.pad(x_adapt, ((0, 0), (0, 0), (1, 1), (1, 1)))
    patches = np.stack(
        [xp[:, :, i:i + Hq, j:j + Wq] for i in range(KK) for j in range(KK)],
        axis=2,
    )
    out = (weights * patches).sum(axis=2) + bias
    return out.astype(np.float32)


def kernel(**inputs):
    global _cached_nc, last_results
    x = np.ascontiguousarray(np.asarray(inputs["x"], np.float32))
    posi_map = np.asarray(inputs["posi_map"], np.float32)

    per_ch = posi_map.reshape(posi_map.shape[0] * posi_map.shape[1], -1)
    if not np.all(per_ch == per_ch[:, :1]):
        # general (spatially varying posi_map) fallback: exact numpy port
        return _numpy_reference(**{k: inputs[k] for k in (
            "x", "offset", "posi_map", "w1", "b1", "w2", "b2", "w_ca", "bias")})

    wts, wb = _host_conv_weights(
        posi_map, inputs["w1"], inputs["b1"], inputs["w2"], inputs["b2"],
        inputs["w_ca"], inputs["bias"],
    )

    variant = os.environ.get("BASS_KERNEL_VARIANT", "dual_v3")
    global _cached_variant
    if _cached_nc is None or _cached_variant != variant:
        if variant == "dual_v3":
            _cached_nc = _build_conv_nc_v3()
        elif variant == "sbuf_v2":
            _cached_nc = _build_conv_nc_v2()
        else:
            _cached_nc = _build_conv_nc(variant)
        _cached_variant = variant

    xpad = np.pad(x, ((0, 0), (0, 0), (1, 1), (1, 1)))
    base_variant = variant[:-4] if variant.endswith("_o16") else variant
    if base_variant in ("bf16_dup3", "sbuf_v2"):
        wts = _pack_dup3(wts)
    elif base_variant == "dual_v3":
        wts = _pack_dup5(wts, wb)
    if base_variant in ("bf16_tiled", "bf16_dup3", "sbuf_v2", "dual_v3"):
        import ml_dtypes

        xpad = xpad.astype(ml_dtypes.bfloat16)
        wts = wts.astype(ml_dtypes.bfloat16)
    if base_variant == "dual_v3":
        in_maps = [{"x": xpad[i], "wts": wts} for i in range(NCORES)]
    else:
        in_maps = [{"x": xpad[i], "wts": wts, "wb": wb} for i in range(NCORES)]
    trace = os.environ.get("BASS_KERNEL_TRACE", "0") == "1"
    if trace:
        _ensure_ntff_hook()
    res = run_bass_kernel_spmd(
        _cached_nc, in_maps, list(range(NCORES)), trace=trace
    )
    last_results = res
    if base_variant == "dual_v3":
        out = np.stack(
            [
                _unscramble_out(np.asarray(res.results[i]["out"], np.float32))
                for i in range(NCORES)
            ],
            axis=0,
        )
    else:
        out = np.stack(
            [np.asarray(res.results[i]["out"], np.float32) for i in range(NCORES)],
            axis=0,
        )
    return out

